# revision 1
# baseline (speedup 1.0000x reference)
"""Trainium2 Bass kernel for nn_HardwiredAttention (NRI-style GNN message passing).

Math (derived from the reference):
  adj[b,t,i,j] = 1/(||locs[b,i,t]-locs[b,j,t]|| + eps) for i!=j, 0 on diag
  out[b,:,t,:] = adj[b,t] @ hidden[b,:,t,:]          ([48,48] @ [48,128] per (b,t))

The rel_rec/rel_send one-hot matmuls in the reference are just gathers over the
fixed fully-connected off-diagonal edge pattern; adj is symmetric.

Distribution: data-parallel over batch, 2 batches per core, 8 cores, no comms.

Per-core layout:
  - elementwise pipeline in partitions p=(s,tau), t=2*tau+s (100 partitions):
    dx/dy from a tiny [100,(c,b,n)] coords tile via stride-0 broadcast APs,
    squares on ACT, d2-add on GPSIMD, sqrt on ACT, (s+eps)+BIGMASK via
    scalar_tensor_tensor, reciprocal_approx_fast on DVE, fp16 convert on ACT.
  - PE transposes [50(tau),48(j)] -> [48(j),50(tau)] per (b,i,s) build a
    block-diagonal fp16 lhsT [96=(s,j), (b,scol,i,tau)].
  - 2-packed matmuls lhsT[96,96] @ hidden[96,128] -> PSUM [96,128] fp32,
    DMA'd straight to HBM.
"""

import os
import sys

sys.path.insert(0, "/opt/trn_rl_repo")

import numpy as np

import bass_rust
import concourse.bass as bass
import concourse.tile as tile
from concourse import bacc, mybir
from concourse.bass_utils import run_bass_kernel_spmd

F32 = mybir.dt.float32
F16 = mybir.dt.float16
ALU = mybir.AluOpType

B, N, T, H = 16, 48, 100, 128
NCORES = 8
BL = B // NCORES          # 2 batches per core
TAU = T // 2              # 50
E = N * N                 # 2304 (full pair matrix incl. diag)
EPS = 1e-5
BIG = 60000.0             # diag mask: 1/(BIG) ~ 1.7e-5 ~ 0 in fp16
GI = 8                    # i's per PSUM transpose group


def _ap(t, offset, dims):
    """Manual access pattern on a tile/tensor handle's underlying tensor."""
    return bass_rust.AP(t.tensor, offset, [list(d) for d in dims])


def build_nc():
    nc = bacc.Bacc("TRN2", target_bir_lowering=False, debug=False)

    # DRAM I/O (per core). Layouts chosen so every DMA is long-contiguous.
    xt = nc.dram_tensor("xt", [2, 128, BL * N], F32, kind="ExternalInput")
    hid = nc.dram_tensor("hid", [BL, 2, N, TAU, H], F16, kind="ExternalInput")
    bm = nc.dram_tensor("bm", [128, E], F16, kind="ExternalInput")
    ident = nc.dram_tensor("ident", [128, TAU], F16, kind="ExternalInput")
    out = nc.dram_tensor("out", [BL, 2, N, TAU, H], F16, kind="ExternalOutput")

    with tile.TileContext(nc) as tc:
        _emit(nc, tc, xt, hid, bm, ident, out)
    nc.compile()
    return nc


def _emit(nc, tc, xt, hid, bm, ident, out):
    FREE = BL * E  # 4608 free elems/partition for pair tiles

    with (
        tc.tile_pool(name="persist", bufs=1) as pp,
        tc.tile_pool(name="tp", bufs=3, space="PSUM") as tp_pool,
        tc.tile_pool(name="mm", bufs=4, space="PSUM") as mm_pool,
        tc.tile_pool(name="ot", bufs=6) as ot_pool,
    ):
        xt_sb = pp.tile([128, 2 * BL * N], F32, tag="xt")
        hid_sb = pp.tile([128, BL * TAU * H], F16, tag="hid")
        bm_sb = pp.tile([128, E], F16, tag="bm")
        id_sb = pp.tile([128, TAU], F16, tag="id")
        dx = pp.tile([128, FREE], F32, tag="dx")
        dy = pp.tile([128, FREE], F32, tag="dy")
        dx2 = pp.tile([128, FREE], F32, tag="dx2")
        dy2 = pp.tile([128, FREE], F32, tag="dy2")
        adj16 = pp.tile([128, FREE], F16, tag="adj16")
        lhsT = pp.tile([128, BL * 2 * N * TAU], F16, tag="lhsT")

        # ---- loads -------------------------------------------------------
        nc.sync.dma_start(xt_sb[:], xt.ap().rearrange("c p q -> p c q"))
        nc.vector.memset(hid_sb[:], 0.0)
        for s in range(2):
            nc.sync.dma_start(
                hid_sb[s * 64 : s * 64 + N, :],
                hid[:, s].rearrange("b j t h -> j b t h"),
            )
        nc.sync.dma_start(bm_sb[:], bm.ap())
        nc.sync.dma_start(id_sb[:], ident.ap())
        # zero the off-diagonal blocks of the block-diag lhsT (whole tensor)
        nc.vector.memset(lhsT[:], 0.0)

        # ---- pairwise distance chain ------------------------------------
        # xt_sb free layout: (c, b, n); strides c:96, b:48, n:1
        def coords_ap(c, vary_i):
            base = c * (BL * N)
            if vary_i:
                dims = [[2 * BL * N, 128], [N, BL], [1, N], [0, N]]
            else:
                dims = [[2 * BL * N, 128], [N, BL], [0, N], [1, N]]
            return _ap(xt_sb[:], base, dims)

        def pair_view(tl):
            return _ap(tl[:], 0, [[FREE, 128], [E, BL], [N, N], [1, N]])

        # chunked over (b, i-half) so the engine chain pipelines
        LFREE = BL * 2 * N * TAU  # 9600
        IH = N // 2               # 24 i's per chunk
        CH = IH * N               # 1152 free elems per chunk
        for b in range(BL):
            for ih in range(2):
                i0 = ih * IH
                off = b * E + i0 * N
                pv = lambda tl: _ap(tl[:], off, [[FREE, 128], [N, IH], [1, N]])
                fl = lambda tl: _ap(tl[:], off, [[FREE, 128], [1, CH]])
                cb = lambda c, vi: _ap(
                    xt_sb[:], c * (BL * N) + b * N + (i0 if vi else 0),
                    [[2 * BL * N, 128], [1, IH], [0, N]] if vi
                    else [[2 * BL * N, 128], [0, IH], [1, N]],
                )
                nc.vector.tensor_tensor(pv(dx), cb(0, True), cb(0, False), ALU.subtract)
                nc.vector.tensor_tensor(pv(dy), cb(1, True), cb(1, False), ALU.subtract)
                nc.scalar.square(fl(dx2), fl(dx))
                nc.scalar.square(fl(dy2), fl(dy))
                nc.gpsimd.tensor_tensor(fl(dx), fl(dx2), fl(dy2), ALU.add)
                nc.scalar.sqrt(fl(dy), fl(dx))
                bm_ap = _ap(bm_sb[:], i0 * N, [[E, 128], [1, CH]])
                nc.vector.scalar_tensor_tensor(
                    fl(dx2), fl(dy), EPS, bm_ap, ALU.add, ALU.add
                )
                nc.vector.reciprocal_approx_fast(out=fl(dy2), in_=fl(dx2))
                nc.scalar.copy(fl(adj16), fl(dy2))

                for s in range(2):
                    for gl in range(IH // GI):
                        g = i0 // GI + gl
                        pt = tp_pool.tile([48, GI * TAU], F16, tag="tp")
                        for ii in range(GI):
                            i = g * GI + ii
                            src = adj16[s * 64 : s * 64 + TAU,
                                        b * E + i * N : b * E + i * N + N]
                            nc.tensor.transpose(
                                pt[:, ii * TAU : (ii + 1) * TAU], src,
                                id_sb[s * 64 : s * 64 + TAU, :]
                            )
                        dst = _ap(
                            lhsT[:],
                            (s * 64) * LFREE + b * (2 * N * TAU) + s * (N * TAU)
                            + g * GI * TAU,
                            [[LFREE, 48], [TAU, GI], [1, TAU]],
                        )
                        csrc = _ap(pt[:], 0, [[GI * TAU, 48], [TAU, GI], [1, TAU]])
                        if g % 2 == 0:
                            nc.vector.tensor_copy(dst, csrc)
                        else:
                            nc.scalar.copy(dst, csrc)

        # ---- packed matmuls + store -------------------------------------
        HF = BL * TAU * H  # hid_sb free size (12800)
        groups = [(tg * 4, min(4, TAU - tg * 4)) for tg in range((TAU + 3) // 4)]
        for b in range(BL):
            for t0, tlen in groups:
                mt = mm_pool.tile([96, 4 * H], F32, tag="mm")
                for k in range(tlen):
                    tau = t0 + k
                    w_ap = _ap(
                        lhsT[:], b * (2 * N * TAU) + tau, [[LFREE, 128], [TAU, 96]]
                    )
                    r_ap = _ap(
                        hid_sb[:], b * (TAU * H) + tau * H, [[HF, 128], [1, H]]
                    )
                    nc.tensor.matmul(
                        mt[:, k * H : (k + 1) * H], w_ap, r_ap,
                        start=True, stop=True,
                    )
                ot = ot_pool.tile([96, 4 * H], F16, tag="ot")
                if t0 % 8 == 0:
                    nc.scalar.copy(ot[:, : tlen * H], mt[:, : tlen * H])
                else:
                    nc.vector.tensor_copy(ot[:, : tlen * H], mt[:, : tlen * H])
                dst = out[b, :, :, t0 : t0 + tlen, :].rearrange(
                    "s i t h -> (s i) (t h)"
                )
                nc.sync.dma_start(dst, ot[:, : tlen * H])


# ----------------------------------------------------------------------------
# Host side
# ----------------------------------------------------------------------------

def _prep_core(locs_c, hidden_c):
    """locs_c [2,48,100,2] f32, hidden_c [2,48,100,128] f32 -> input map."""
    lc = locs_c.reshape(BL, N, TAU, 2, 2)            # (b, n, tau, s, c)
    xt_d = lc.transpose(4, 3, 2, 0, 1).reshape(2, 2, TAU, BL * N)  # (c, s, tau, q)
    xt = np.zeros((2, 128, BL * N), dtype=np.float32)
    xt[:, 0:TAU] = xt_d[:, 0]
    xt[:, 64 : 64 + TAU] = xt_d[:, 1]
    # filler rows: spread points (x=n, y=0) so junk weights stay finite
    fill = np.tile(np.arange(N, dtype=np.float32), BL)[None, :]
    xt[0, TAU:64] = fill
    xt[0, 64 + TAU : 128] = fill
    hc = hidden_c.astype(np.float16).reshape(BL, N, TAU, 2, H)
    hid = np.ascontiguousarray(hc.transpose(0, 3, 1, 2, 4))  # (b, s, j, tau, h)
    return {"xt": xt, "hid": hid}


_BM = None
_IDENT = None


def _consts():
    global _BM, _IDENT
    if _BM is None:
        row = (BIG * np.eye(N, dtype=np.float32)).astype(np.float16).reshape(1, E)
        _BM = np.ascontiguousarray(np.repeat(row, 128, axis=0))
        idm = np.zeros((128, TAU), dtype=np.float16)
        idm[0:TAU] = np.eye(TAU, dtype=np.float16)
        idm[64 : 64 + TAU] = np.eye(TAU, dtype=np.float16)
        _IDENT = idm
    return _BM, _IDENT


_NC = None
LAST_EXEC_NS = None


def _get_nc():
    global _NC
    if _NC is None:
        _NC = build_nc()
    return _NC


def kernel(locs, hidden, rel_rec=None, rel_send=None):
    locs = np.asarray(locs, dtype=np.float32)
    hidden = np.asarray(hidden, dtype=np.float32)
    bm, ident = _consts()
    in_maps = []
    for k in range(NCORES):
        m = _prep_core(locs[2 * k : 2 * k + 2], hidden[2 * k : 2 * k + 2])
        m["bm"] = bm
        m["ident"] = ident
        in_maps.append(m)

    nc = _get_nc()
    import kernel as _self
    trace = False
    res = run_bass_kernel_spmd(nc, in_maps, list(range(NCORES)), trace=trace)
    _self.LAST_EXEC_NS = getattr(res, "exec_time_ns", None)
    outs = []
    for k in range(NCORES):
        o = res.results[k]["out"].astype(np.float32).reshape(BL, 2, N, TAU, H)
        o = o.transpose(0, 2, 3, 1, 4).reshape(BL, N, T, H)  # t = 2*tau+s
        outs.append(o)
    return np.ascontiguousarray(np.concatenate(outs, axis=0), dtype=np.float32)


if __name__ == "__main__":
    # smoke test with random data against a local numpy reference
    rng = np.random.default_rng(0)
    locs = rng.standard_normal((B, N, T, 2), dtype=np.float32)
    hidden = rng.standard_normal((B, N, T, H), dtype=np.float32)
    got = kernel(locs, hidden)
    x = locs[..., 0]
    y = locs[..., 1]
    d = np.sqrt((x[:, :, None] - x[:, None]) ** 2 + (y[:, :, None] - y[:, None]) ** 2)
    w = 1.0 / (d + EPS) * (1.0 - np.eye(N)[None, :, :, None])
    want = np.einsum("bijt,bjth->bith", w.astype(np.float32), hidden)
    err = np.linalg.norm(got - want) / np.linalg.norm(want)
    print("rel err vs numpy:", err)



# revision 2
# speedup vs baseline: 26103.1165x; 26103.1165x over previous
"""Trainium2 Bass kernel for nn_HardwiredAttention (NRI-style GNN message passing).

Math (derived from the reference):
  adj[b,t,i,j] = 1/(||locs[b,i,t]-locs[b,j,t]|| + eps) for i!=j, 0 on diag
  out[b,:,t,:] = adj[b,t] @ hidden[b,:,t,:]          ([48,48] @ [48,128] per (b,t))

The rel_rec/rel_send one-hot matmuls in the reference are just gathers over the
fixed fully-connected off-diagonal edge pattern; adj is symmetric.

Distribution: data-parallel over batch, 2 batches per core, 8 cores, no comms.

Per-core layout:
  - elementwise pipeline in partitions p=(s,tau), t=2*tau+s (100 partitions):
    dx/dy from a tiny [100,(c,b,n)] coords tile via stride-0 broadcast APs,
    squares on ACT, d2-add on GPSIMD, sqrt on ACT, (s+eps)+BIGMASK via
    scalar_tensor_tensor, reciprocal_approx_fast on DVE, fp16 convert on ACT.
  - PE transposes [50(tau),48(j)] -> [48(j),50(tau)] per (b,i,s) build a
    block-diagonal fp16 lhsT [96=(s,j), (b,scol,i,tau)].
  - 2-packed matmuls lhsT[96,96] @ hidden[96,128] -> PSUM [96,128] fp32,
    DMA'd straight to HBM.
"""

import os
import sys

sys.path.insert(0, "/opt/trn_rl_repo")

import numpy as np

import bass_rust
import concourse.bass as bass
import concourse.tile as tile
from concourse import bacc, mybir
from concourse.bass_utils import run_bass_kernel_spmd

F32 = mybir.dt.float32
F16 = mybir.dt.float16
ALU = mybir.AluOpType

B, N, T, H = 16, 48, 100, 128
NCORES = 8
BL = B // NCORES          # 2 batches per core
TAU = T // 2              # 50
E = N * N                 # 2304 (full pair matrix incl. diag)
EPS = 1e-5
BIG = 60000.0             # diag mask: 1/(BIG) ~ 1.7e-5 ~ 0 in fp16
GI = 8                    # i's per PSUM transpose group


def _ap(t, offset, dims):
    """Manual access pattern on a tile/tensor handle's underlying tensor."""
    return bass_rust.AP(t.tensor, offset, [list(d) for d in dims])


def build_nc():
    nc = bacc.Bacc("TRN2", target_bir_lowering=False, debug=False)

    # DRAM I/O (per core). Layouts chosen so every DMA is long-contiguous.
    xt = nc.dram_tensor("xt", [2, 128, BL * N], F32, kind="ExternalInput")
    hid = nc.dram_tensor("hid", [BL, 2, N, TAU, H], F16, kind="ExternalInput")
    bm = nc.dram_tensor("bm", [128, E], F16, kind="ExternalInput")
    ident = nc.dram_tensor("ident", [128, TAU], F16, kind="ExternalInput")
    out = nc.dram_tensor("out", [BL, 2, N, TAU, H], F16, kind="ExternalOutput")

    with tile.TileContext(nc) as tc:
        _emit(nc, tc, xt, hid, bm, ident, out)
    nc.compile()
    return nc


def _emit(nc, tc, xt, hid, bm, ident, out):
    FREE = BL * E  # 4608 free elems/partition for pair tiles

    with (
        tc.tile_pool(name="persist", bufs=1) as pp,
        tc.tile_pool(name="tp", bufs=3, space="PSUM") as tp_pool,
        tc.tile_pool(name="mm", bufs=4, space="PSUM") as mm_pool,
        tc.tile_pool(name="ot", bufs=6) as ot_pool,
    ):
        xt_sb = pp.tile([128, 2 * BL * N], F32, tag="xt")
        hid_sb = pp.tile([128, BL * TAU * H], F16, tag="hid")
        bm_sb = pp.tile([128, E], F16, tag="bm")
        id_sb = pp.tile([128, TAU], F16, tag="id")
        dx = pp.tile([128, FREE], F32, tag="dx")
        dy = pp.tile([128, FREE], F32, tag="dy")
        dx2 = pp.tile([128, FREE], F32, tag="dx2")
        dy2 = pp.tile([128, FREE], F32, tag="dy2")
        adj16 = pp.tile([128, FREE], F16, tag="adj16")
        lhsT = pp.tile([128, BL * 2 * N * TAU], F16, tag="lhsT")

        # ---- loads -------------------------------------------------------
        nc.sync.dma_start(xt_sb[:], xt.ap().rearrange("c p q -> p c q"))
        nc.vector.memset(hid_sb[:], 0.0)
        for s in range(2):
            nc.sync.dma_start(
                hid_sb[s * 64 : s * 64 + N, :],
                hid[:, s].rearrange("b j t h -> j b t h"),
            )
        nc.sync.dma_start(bm_sb[:], bm.ap())
        nc.sync.dma_start(id_sb[:], ident.ap())
        # zero the off-diagonal blocks of the block-diag lhsT (whole tensor)
        nc.vector.memset(lhsT[:], 0.0)

        # ---- pairwise distance chain ------------------------------------
        # xt_sb free layout: (c, b, n); strides c:96, b:48, n:1
        def coords_ap(c, vary_i):
            base = c * (BL * N)
            if vary_i:
                dims = [[2 * BL * N, 128], [N, BL], [1, N], [0, N]]
            else:
                dims = [[2 * BL * N, 128], [N, BL], [0, N], [1, N]]
            return _ap(xt_sb[:], base, dims)

        def pair_view(tl):
            return _ap(tl[:], 0, [[FREE, 128], [E, BL], [N, N], [1, N]])

        # chunked over (b, i-half) so the engine chain pipelines
        LFREE = BL * 2 * N * TAU  # 9600
        IH = N // 2               # 24 i's per chunk
        CH = IH * N               # 1152 free elems per chunk
        for b in range(BL):
            for ih in range(2):
                i0 = ih * IH
                off = b * E + i0 * N
                pv = lambda tl: _ap(tl[:], off, [[FREE, 128], [N, IH], [1, N]])
                fl = lambda tl: _ap(tl[:], off, [[FREE, 128], [1, CH]])
                cb = lambda c, vi: _ap(
                    xt_sb[:], c * (BL * N) + b * N + (i0 if vi else 0),
                    [[2 * BL * N, 128], [1, IH], [0, N]] if vi
                    else [[2 * BL * N, 128], [0, IH], [1, N]],
                )
                nc.vector.tensor_tensor(pv(dx), cb(0, True), cb(0, False), ALU.subtract)
                nc.vector.tensor_tensor(pv(dy), cb(1, True), cb(1, False), ALU.subtract)
                nc.scalar.square(fl(dx2), fl(dx))
                nc.scalar.square(fl(dy2), fl(dy))
                nc.gpsimd.tensor_tensor(fl(dx), fl(dx2), fl(dy2), ALU.add)
                nc.scalar.sqrt(fl(dy), fl(dx))
                bm_ap = _ap(bm_sb[:], i0 * N, [[E, 128], [1, CH]])
                nc.vector.scalar_tensor_tensor(
                    fl(dx2), fl(dy), EPS, bm_ap, ALU.add, ALU.add
                )
                nc.vector.reciprocal_approx_fast(out=fl(dy2), in_=fl(dx2))
                nc.scalar.copy(fl(adj16), fl(dy2))

                for s in range(2):
                    for gl in range(IH // GI):
                        g = i0 // GI + gl
                        pt = tp_pool.tile([48, GI * TAU], F16, tag="tp")
                        for ii in range(GI):
                            i = g * GI + ii
                            src = adj16[s * 64 : s * 64 + TAU,
                                        b * E + i * N : b * E + i * N + N]
                            nc.tensor.transpose(
                                pt[:, ii * TAU : (ii + 1) * TAU], src,
                                id_sb[s * 64 : s * 64 + TAU, :]
                            )
                        dst = _ap(
                            lhsT[:],
                            (s * 64) * LFREE + b * (2 * N * TAU) + s * (N * TAU)
                            + g * GI * TAU,
                            [[LFREE, 48], [TAU, GI], [1, TAU]],
                        )
                        csrc = _ap(pt[:], 0, [[GI * TAU, 48], [TAU, GI], [1, TAU]])
                        if g % 2 == 0:
                            nc.vector.tensor_copy(dst, csrc)
                        else:
                            nc.scalar.copy(dst, csrc)

        # ---- packed matmuls + store -------------------------------------
        HF = BL * TAU * H  # hid_sb free size (12800)
        groups = [(tg * 4, min(4, TAU - tg * 4)) for tg in range((TAU + 3) // 4)]
        for b in range(BL):
            for t0, tlen in groups:
                mt = mm_pool.tile([96, 4 * H], F32, tag="mm")
                for k in range(tlen):
                    tau = t0 + k
                    w_ap = _ap(
                        lhsT[:], b * (2 * N * TAU) + tau, [[LFREE, 128], [TAU, 96]]
                    )
                    r_ap = _ap(
                        hid_sb[:], b * (TAU * H) + tau * H, [[HF, 128], [1, H]]
                    )
                    nc.tensor.matmul(
                        mt[:, k * H : (k + 1) * H], w_ap, r_ap,
                        start=True, stop=True,
                    )
                ot = ot_pool.tile([96, 4 * H], F16, tag="ot")
                if t0 % 8 == 0:
                    nc.scalar.copy(ot[:, : tlen * H], mt[:, : tlen * H])
                else:
                    nc.vector.tensor_copy(ot[:, : tlen * H], mt[:, : tlen * H])
                dst = out[b, :, :, t0 : t0 + tlen, :].rearrange(
                    "s i t h -> (s i) (t h)"
                )
                nc.sync.dma_start(dst, ot[:, : tlen * H])


# ----------------------------------------------------------------------------
# Host side
# ----------------------------------------------------------------------------

def _prep_core(locs_c, hidden_c):
    """locs_c [2,48,100,2] f32, hidden_c [2,48,100,128] f32 -> input map."""
    lc = locs_c.reshape(BL, N, TAU, 2, 2)            # (b, n, tau, s, c)
    xt_d = lc.transpose(4, 3, 2, 0, 1).reshape(2, 2, TAU, BL * N)  # (c, s, tau, q)
    xt = np.zeros((2, 128, BL * N), dtype=np.float32)
    xt[:, 0:TAU] = xt_d[:, 0]
    xt[:, 64 : 64 + TAU] = xt_d[:, 1]
    # filler rows: spread points (x=n, y=0) so junk weights stay finite
    fill = np.tile(np.arange(N, dtype=np.float32), BL)[None, :]
    xt[0, TAU:64] = fill
    xt[0, 64 + TAU : 128] = fill
    hc = hidden_c.astype(np.float16).reshape(BL, N, TAU, 2, H)
    hid = np.ascontiguousarray(hc.transpose(0, 3, 1, 2, 4))  # (b, s, j, tau, h)
    return {"xt": xt, "hid": hid}


_BM = None
_IDENT = None


def _consts():
    global _BM, _IDENT
    if _BM is None:
        row = (BIG * np.eye(N, dtype=np.float32)).astype(np.float16).reshape(1, E)
        _BM = np.ascontiguousarray(np.repeat(row, 128, axis=0))
        idm = np.zeros((128, TAU), dtype=np.float16)
        idm[0:TAU] = np.eye(TAU, dtype=np.float16)
        idm[64 : 64 + TAU] = np.eye(TAU, dtype=np.float16)
        _IDENT = idm
    return _BM, _IDENT


_NC = None
LAST_EXEC_NS = None


def _get_nc():
    global _NC
    if _NC is None:
        _NC = build_nc()
    return _NC


def kernel(locs, hidden, rel_rec=None, rel_send=None):
    locs = np.asarray(locs, dtype=np.float32)
    hidden = np.asarray(hidden, dtype=np.float32)
    bm, ident = _consts()
    in_maps = []
    for k in range(NCORES):
        m = _prep_core(locs[2 * k : 2 * k + 2], hidden[2 * k : 2 * k + 2])
        m["bm"] = bm
        m["ident"] = ident
        in_maps.append(m)

    nc = _get_nc()
    import kernel as _self
    trace = False
    res = run_bass_kernel_spmd(nc, in_maps, list(range(NCORES)), trace=trace)
    _self.LAST_RES = res
    _self.LAST_EXEC_NS = getattr(res, "exec_time_ns", None)
    outs = []
    for k in range(NCORES):
        o = res.results[k]["out"].astype(np.float32).reshape(BL, 2, N, TAU, H)
        o = o.transpose(0, 2, 3, 1, 4).reshape(BL, N, T, H)  # t = 2*tau+s
        outs.append(o)
    return np.ascontiguousarray(np.concatenate(outs, axis=0), dtype=np.float32)


if __name__ == "__main__":
    # smoke test with random data against a local numpy reference
    rng = np.random.default_rng(0)
    locs = rng.standard_normal((B, N, T, 2), dtype=np.float32)
    hidden = rng.standard_normal((B, N, T, H), dtype=np.float32)
    got = kernel(locs, hidden)
    x = locs[..., 0]
    y = locs[..., 1]
    d = np.sqrt((x[:, :, None] - x[:, None]) ** 2 + (y[:, :, None] - y[:, None]) ** 2)
    w = 1.0 / (d + EPS) * (1.0 - np.eye(N)[None, :, :, None])
    want = np.einsum("bijt,bjth->bith", w.astype(np.float32), hidden)
    err = np.linalg.norm(got - want) / np.linalg.norm(want)
    print("rel err vs numpy:", err)



# revision 21
# speedup vs baseline: 29326.1777x; 1.1235x over previous
"""Trainium2 Bass kernel for nn_HardwiredAttention (NRI-style GNN message passing).

Math (derived from the reference):
  adj[b,t,i,j] = 1/(||locs[b,i,t]-locs[b,j,t]|| + eps) for i!=j, 0 on diag
  out[b,:,t,:] = adj[b,t] @ hidden[b,:,t,:]          ([48,48] @ [48,128] per (b,t))

Distribution: data-parallel over batch, 2 batches per core, 8 cores, no comms.

Per-core design (v2):
  - partitions p=(s,tau), t=2*tau+s, rows p=s*50+tau (100 used).
  - pairwise chain in fp32 (exact subtract; d2 can be ~1e-8 so fp16 is unsafe):
    sub_x on DVE, sub_y on GPSIMD, squares on ACT, d2-add on DVE,
    +BIG on the 96 diag elems (tiny strided tensor_scalar), sqrt on ACT,
    +eps (ACT add / DVE ts), reciprocal_approx_fast on DVE -> fp16 adj16.
  - PE transposes [50,48] -> [48,50] per (b,s,i) into PSUM, copied into a
    block-diag fp16 lhsT [96=(s,j), (b,scol,i,tau)]; zero blocks DMA'd once.
  - matmuls lhsT[96,96] @ hid[96,128] -> fp16 PSUM, drained by fp16 2x-mode
    copies split over DVE/ACT/GPSIMD, DMA'd to HBM.
"""

import os
import sys

sys.path.insert(0, "/opt/trn_rl_repo")

import numpy as np

import bass_rust
import concourse.bass as bass
import concourse.tile as tile
from concourse import bacc, mybir
from concourse.bass_utils import run_bass_kernel_spmd

F32 = mybir.dt.float32
F16 = mybir.dt.float16
ALU = mybir.AluOpType

B, N, T, H = 16, 48, 100, 128
NCORES = 8
BL = B // NCORES          # 2 batches per core
TAU = T // 2              # 50
E = N * N                 # 2304 pair block per batch
EPS = 1e-5
BIG = 1e8                 # added to diag of d2: w_diag = 1/(1e4+eps) ~ 1e-4
IH = N // 2               # 24 i's per chunk
CH = IH * N               # 1152 free elems per chunk
PITCH = BL * E            # 4608 free elems/partition for pair tiles
LF = BL * 2 * N * TAU     # 9600 lhsT free elems/row
HF = BL * TAU * H         # 12800 hid free elems/row


def _ap(t, offset, dims):
    return bass_rust.AP(t.tensor, offset, [list(d) for d in dims])


def build_nc():
    nc = bacc.Bacc("TRN2", target_bir_lowering=False, debug=False)

    xt = nc.dram_tensor("xt", [2, 128, BL * N], F32, kind="ExternalInput")
    hid = nc.dram_tensor("hid", [128, HF], F16, kind="ExternalInput")
    ident = nc.dram_tensor("ident", [128, TAU], F16, kind="ExternalInput")
    zoff = nc.dram_tensor("zoff", [N, BL, N * TAU], F16, kind="ExternalInput")
    zrow = nc.dram_tensor("zrow", [16, LF], F16, kind="ExternalInput")
    out = nc.dram_tensor("out", [BL, 2, N, TAU, H], F16, kind="ExternalOutput")

    with tile.TileContext(nc) as tc:
        _emit(nc, tc, xt, hid, ident, zoff, zrow, out)
    nc.compile()
    return nc


def _emit(nc, tc, xt, hid, ident, zoff, zrow, out):
    with (
        tc.tile_pool(name="persist", bufs=1) as pp,
        tc.tile_pool(name="tp", bufs=2, space="PSUM") as tp_pool,
        tc.tile_pool(name="mm", bufs=3, space="PSUM") as mm_pool,
        tc.tile_pool(name="ot", bufs=4) as ot_pool,
    ):
        xt_sb = pp.tile([128, 2 * BL * N], F32, tag="xt")
        hid_sb = pp.tile([128, HF], F16, tag="hid")
        id_sb = pp.tile([128, TAU], F16, tag="id")
        dx = pp.tile([128, PITCH], F32, tag="dx")
        dy = pp.tile([128, PITCH], F32, tag="dy")
        dx2 = pp.tile([128, PITCH], F32, tag="dx2")
        dy2 = pp.tile([128, PITCH], F32, tag="dy2")
        adj16 = pp.tile([128, PITCH], F16, tag="adj16")
        lhsT = pp.tile([128, LF], F16, tag="lhsT")

        # ---- loads -------------------------------------------------------
        nc.sync.dma_start(xt_sb[:], xt.ap().rearrange("c p q -> p c q"))
        nc.sync.dma_start(hid_sb[:], hid.ap())
        nc.sync.dma_start(id_sb[:], ident.ap())
        # zero lhsT: off-diagonal blocks of data rows + the junk rows
        # (rows 48-63, 112-127) that K=128 matmuls read against hid zeros
        nc.sync.dma_start(
            _ap(lhsT[:], N * TAU,
                [[LF, N], [2 * N * TAU, BL], [1, N * TAU]]),
            zoff.ap(),
        )
        nc.sync.dma_start(
            _ap(lhsT[:], 64 * LF,
                [[LF, N], [2 * N * TAU, BL], [1, N * TAU]]),
            zoff.ap(),
        )
        nc.sync.dma_start(_ap(lhsT[:], 48 * LF, [[LF, 16], [1, LF]]), zrow.ap())
        nc.sync.dma_start(_ap(lhsT[:], 112 * LF, [[LF, 16], [1, LF]]), zrow.ap())

        # ---- helpers -----------------------------------------------------
        # free layout inside a batch block: b=0 -> (i, j) ; b=1 -> (j, i)
        def chunk_ap(t, b, i0):
            base = b * E
            if b == 0:
                return _ap(t[:], base + i0 * N, [[PITCH, 128], [1, CH]])
            return _ap(t[:], base + i0, [[PITCH, 128], [N, N], [1, IH]])

        def coord_aps(b, i0):
            # returns (xi_x, xj_x, xi_y, xj_y) matching chunk iteration order
            res = []
            for c in range(2):
                cb = c * (BL * N) + b * N
                if b == 0:
                    xi = _ap(xt_sb[:], cb + i0, [[2 * BL * N, 128], [1, IH], [0, N]])
                    xj = _ap(xt_sb[:], cb, [[2 * BL * N, 128], [0, IH], [1, N]])
                else:
                    xi = _ap(xt_sb[:], cb + i0, [[2 * BL * N, 128], [0, N], [1, IH]])
                    xj = _ap(xt_sb[:], cb, [[2 * BL * N, 128], [1, N], [0, IH]])
                res += [xi, xj]
            return res

        def diag_ap(b, i0):
            return _ap(dx[:], b * E + i0 * (N + 1), [[PITCH, 128], [N + 1, IH]])

        # ---- per-batch pipeline ------------------------------------------
        tgroups = [(g * 8, min(8, TAU - g * 8)) for g in range((TAU + 7) // 8)]

        def cp_vec(dst, src):
            nc.vector.tensor_copy(dst, src)

        def cp_act(dst, src):
            nc.scalar.copy(dst, src)

        def cp_gps(dst, src):
            nc.gpsimd.tensor_copy(dst, src)

        # GPSIMD cannot read PSUM; PSUM-sourced copies go to DVE/ACT only
        ocopy_engines = [cp_vec, cp_act]
        lcopy_engines = [cp_vec, cp_act]
        oc = 0
        lc = 0

        for b in range(BL):
            # stage 1: subtract + squares for both chunks
            for ih in range(2):
                i0 = ih * IH
                xi_x, xj_x, xi_y, xj_y = coord_aps(b, i0)
                cdx = chunk_ap(dx, b, i0)
                cdy = chunk_ap(dy, b, i0)
                nc.vector.tensor_tensor(cdx, xi_x, xj_x, ALU.subtract)
                nc.gpsimd.tensor_tensor(cdy, xi_y, xj_y, ALU.subtract)
                nc.scalar.square(chunk_ap(dx2, b, i0), cdx)
                nc.scalar.square(chunk_ap(dy2, b, i0), cdy)

            # stage 2: d2, diag, sqrt, +eps, recip, transpose, lhsT
            for ih in range(2):
                i0 = ih * IH
                cdx = chunk_ap(dx, b, i0)
                cdy = chunk_ap(dy, b, i0)
                cdx2 = chunk_ap(dx2, b, i0)
                cdy2 = chunk_ap(dy2, b, i0)
                cadj = chunk_ap(adj16, b, i0)
                nc.vector.tensor_tensor(cdx, cdx2, cdy2, ALU.add)   # d2 -> dx
                nc.vector.tensor_scalar_add(diag_ap(b, i0), diag_ap(b, i0), BIG)
                nc.scalar.sqrt(cdy, cdx)                            # d -> dy
                if b == 0:
                    nc.scalar.activation(                           # d+eps -> dx2
                        cdx2, cdy, mybir.ActivationFunctionType.Copy, bias=EPS
                    )
                else:
                    nc.vector.tensor_scalar_add(cdx2, cdy, EPS)
                nc.vector.reciprocal_approx_fast(out=cdy2, in_=cdx2)
                if b == 0:
                    nc.scalar.copy(cadj, cdy2)                      # fp32 -> fp16
                else:
                    nc.gpsimd.tensor_copy(cadj, cdy2)

                GI = 12  # i's per PSUM transpose tile (must fit one 2KB bank)
                for s in range(2):
                    for g in range(IH // GI):
                        i0g = i0 + g * GI
                        pt = tp_pool.tile([N, GI * TAU], F16, tag="tp")
                        for ii in range(GI):
                            i = i0g + ii
                            if b == 0:
                                src = adj16[s * 64 : s * 64 + TAU,
                                            i * N : (i + 1) * N]
                            else:
                                src = _ap(adj16[:], (s * 64) * PITCH + E + i,
                                          [[PITCH, TAU], [N, N]])
                            nc.tensor.transpose(
                                pt[:, ii * TAU : (ii + 1) * TAU], src,
                                id_sb[s * 64 : s * 64 + TAU, :],
                            )
                        dst = _ap(
                            lhsT[:],
                            (s * 64) * LF + b * (2 * N * TAU) + s * (N * TAU)
                            + i0g * TAU,
                            [[LF, N], [1, GI * TAU]],
                        )
                        csrc = _ap(pt[:], 0, [[GI * TAU, N], [1, GI * TAU]])
                        lcopy_engines[lc % 2](dst, csrc)
                        lc += 1

            # stage 3: matmuls + drain + store
            for t0, tlen in tgroups:
                mt = mm_pool.tile([2 * N, 8 * H], F32, tag="mm")
                for k in range(tlen):
                    tau = t0 + k
                    w_ap = _ap(lhsT[:], b * (2 * N * TAU) + tau,
                               [[LF, 128], [TAU, 2 * N]])
                    r_ap = _ap(hid_sb[:], b * (TAU * H) + tau * H,
                               [[HF, 128], [1, H]])
                    nc.tensor.matmul(
                        mt[:, k * H : (k + 1) * H], w_ap, r_ap,
                        start=True, stop=True,
                    )
                ot = ot_pool.tile([2 * N, 8 * H], F16, tag="ot")
                cp = ocopy_engines[oc % 2]
                oc += 1
                cp(ot[:, : tlen * H], mt[:, : tlen * H])
                dst = out[b, :, :, t0 : t0 + tlen, :].rearrange(
                    "s i t h -> (s i) (t h)"
                )
                nc.sync.dma_start(dst, ot[:, : tlen * H])


# ----------------------------------------------------------------------------
# Host side
# ----------------------------------------------------------------------------

def _prep_core(locs_c, hidden_c):
    """locs_c [2,48,100,2] f32, hidden_c [2,48,100,128] f32 -> input map."""
    lc = locs_c.reshape(BL, N, TAU, 2, 2)                  # (b, n, tau, s, c)
    xt_d = lc.transpose(4, 3, 2, 0, 1).reshape(2, 2, TAU, BL * N)  # (c,s,tau,q)
    xt = np.zeros((2, 128, BL * N), dtype=np.float32)
    xt[:, 0:TAU] = xt_d[:, 0]
    xt[:, 64 : 64 + TAU] = xt_d[:, 1]
    # filler rows: spread points (x=n, y=0) so junk weights stay finite
    fill = np.tile(np.arange(N, dtype=np.float32), BL)[None, :]
    xt[0, TAU:64] = fill
    xt[0, 64 + TAU : 128] = fill
    hc = hidden_c.astype(np.float16).reshape(BL, N, TAU, 2, H)
    hjb = hc.transpose(3, 1, 0, 2, 4)                      # (s, j, b, tau, h)
    hid = np.zeros((128, HF), dtype=np.float16)
    for s in range(2):
        hid[s * 64 : s * 64 + N] = hjb[s].reshape(N, HF)
    return {"xt": xt, "hid": hid}


_IDENT = None
_ZEROS = None


def _consts():
    global _IDENT, _ZEROS
    if _IDENT is None:
        idm = np.zeros((128, TAU), dtype=np.float16)
        idm[0:TAU] = np.eye(TAU, dtype=np.float16)
        idm[64 : 64 + TAU] = np.eye(TAU, dtype=np.float16)
        _IDENT = idm
        _ZEROS = (np.zeros((N, BL, N * TAU), dtype=np.float16),
                  np.zeros((16, LF), dtype=np.float16))
    return _IDENT, _ZEROS


_NC = None
LAST_EXEC_NS = None
LAST_RES = None


def _get_nc():
    global _NC
    if _NC is None:
        _NC = build_nc()
    return _NC


def kernel(locs, hidden, rel_rec=None, rel_send=None):
    locs = np.asarray(locs, dtype=np.float32)
    hidden = np.asarray(hidden, dtype=np.float32)
    ident, (zoff, zrow) = _consts()
    in_maps = []
    for k in range(NCORES):
        m = _prep_core(locs[2 * k : 2 * k + 2], hidden[2 * k : 2 * k + 2])
        m["ident"] = ident
        m["zoff"] = zoff
        m["zrow"] = zrow
        in_maps.append(m)

    nc = _get_nc()
    import kernel as _self
    res = run_bass_kernel_spmd(nc, in_maps, list(range(NCORES)), trace=False)
    _self.LAST_RES = res
    _self.LAST_EXEC_NS = getattr(res, "exec_time_ns", None)
    outs = []
    for k in range(NCORES):
        o = res.results[k]["out"].astype(np.float32).reshape(BL, 2, N, TAU, H)
        o = o.transpose(0, 2, 3, 1, 4).reshape(BL, N, T, H)  # t = 2*tau+s
        outs.append(o)
    return np.ascontiguousarray(np.concatenate(outs, axis=0), dtype=np.float32)


if __name__ == "__main__":
    rng = np.random.default_rng(0)
    locs = rng.standard_normal((B, N, T, 2), dtype=np.float32)
    hidden = rng.standard_normal((B, N, T, H), dtype=np.float32)
    got = kernel(locs, hidden)
    x = locs[..., 0]
    y = locs[..., 1]
    d = np.sqrt((x[:, :, None] - x[:, None]) ** 2 + (y[:, :, None] - y[:, None]) ** 2)
    w = 1.0 / (d + EPS) * (1.0 - np.eye(N)[None, :, :, None])
    want = np.einsum("bijt,bjth->bith", w.astype(np.float32), hidden)
    err = np.linalg.norm(got - want) / np.linalg.norm(want)
    print("rel err vs numpy:", err)
